# revision 25
# baseline (speedup 1.0000x reference)
"""Block-circulant linear layer on 8 Trainium2 NeuronCores.

Math: y[n, (j,b)] = sum_i circconv(x[n,i,:], c[j,i,:])[b] + bias.
Factorized via packed-real-FFT (halfcomplex, 128 slots of (re,im)):

  stage A (rfft):    t1 = F^T @ x^T        512-col MMs, bh-major xt layout
  permute A->B:      32 DMAs/chunk, stride-32 partition reads (4 port octets)
  stage B (mixing):  t2 = W2_g^T @ t1p     block-diagonal per slot-group g
  permute B->C:      DRAM round-trip (1 write + 4 permuting reads) so it runs
                     on the HBM channel in parallel with A->B's SBUF crossbar
  stage C (irfft):   G stationary, t2p moving (512-col j-pairs); bias fused
                     into the DVE evacuation; feature-major y + host unpack

All matmuls bf16 (f32 PSUM). The whole chunk loop is software-pipelined:
stage s of chunk n is emitted at step n+s, so each in-order engine stream
interleaves A(n) / B(n-1) / C(n-2) and never stalls on its own chunk's
evac+permute chain. Sharding: data-parallel, 1024 tokens per core; weights
replicated. Host: transpose+chunk x shards, build F/W2/G/bias layouts,
unpack feature-major y.

Perf notes (measured): SBUF partition-crossing DMA runs at only ~70-130 GB/s
(crossbar-limited), so the two 2MB/chunk permutes dominated; splitting them
across the crossbar (A->B) and HBM (B->C) channels roughly halved HW time.
"""

import numpy as np

try:
    import ml_dtypes
    _BF16 = ml_dtypes.bfloat16
except ImportError:  # pragma: no cover
    _BF16 = None

MID_BF16 = True

BLOCK = 256
NB = 16          # in/out blocks
NSLOT = 128      # frequency slots (halfcomplex pairs)
N_CORES = 8
TOK_PER_CORE = 1024
CHUNK = 256      # tokens per pipeline chunk
N_CHUNKS = TOK_PER_CORE // CHUNK
IN_F = NB * BLOCK  # 4096


def _build_weights(c: np.ndarray, strided: bool = True):
    """Host-side weight construction (float64 for accuracy, cast to f32).

    strided=True: permute DMAs read stride-32 partition slices (4 distinct
    SBUF port octets per DMA -> ~4x read bandwidth). t1 partitions are then
    sigma(k)=32s+g (F columns reordered) and t2 partitions are 32s+q
    (block-diagonal W2). strided=False keeps contiguous 4-partition reads
    (t1 partition = k, t2 partition = 4q+s) for CoreSim validation.
    """
    B, K = BLOCK, NSLOT
    b = np.arange(B)
    k = np.arange(K)
    theta = 2 * np.pi * np.outer(b, k) / B
    F_re = np.cos(theta)
    F_im = -np.sin(theta)
    F_im[:, 0] = (-1.0) ** b            # Nyquist column in the c=1 half, k=0
    F_pack = np.concatenate([F_re, F_im], axis=1)   # [256 b, 256 (c,k)]

    G_re = np.zeros((K, B))
    G_im = np.zeros((K, B))
    kk = np.arange(1, K)
    th = 2 * np.pi * np.outer(kk, b) / B
    G_re[1:] = 2.0 * np.cos(th) / B
    G_re[0] = 1.0 / B
    G_im[1:] = -2.0 * np.sin(th) / B
    G_im[0] = ((-1.0) ** b) / B
    G_pack = np.stack([G_re, G_im], axis=0)          # [2, 128 k, 256 b]

    Cf = np.fft.rfft(c.astype(np.float64), axis=-1)  # [j, i, 129]
    A = Cf.real
    Bm = Cf.imag
    # W2[g]: rows (s,ch,i) = 32s+16ch+i -> cols: strided ? 32s+q : 4q+s
    # (q = 16ch'+j). The column order fixes stage B's output partition index,
    # which in turn decides the B->C permute's source-partition pattern.
    W2 = np.zeros((32, 128, 128))
    for g in range(32):
        for s in range(4):
            ks = 4 * g + s
            blk = np.zeros((32, 32))                 # rows (c,i) -> cols (c',j)
            if ks == 0:
                blk[0:16, 0:16] = A[:, :, 0].T
                blk[16:32, 16:32] = A[:, :, 128].T
            else:
                a = A[:, :, ks].T
                bb = Bm[:, :, ks].T
                blk[0:16, 0:16] = a
                blk[16:32, 0:16] = -bb
                blk[0:16, 16:32] = bb
                blk[16:32, 16:32] = a
            if strided:
                W2[g, 32 * s:32 * s + 32, 32 * s:32 * s + 32] = blk
            else:
                W2[g, 32 * s:32 * s + 32, s::4] = blk

    sigma = np.empty(128, dtype=np.int64)
    for g in range(32):
        for s in range(4):
            sigma[32 * s + g] = 4 * g + s

    f_host = (
        F_pack.reshape(2, 128, 2, 128).transpose(1, 0, 2, 3).reshape(128, 512)
    )  # [p=b_local, bh*256 + ch*128 + k]
    if strided:
        # t1 partition order sigma(k) = 32s+g: reorder F's k-columns
        f_host = np.ascontiguousarray(
            f_host.reshape(128, 4, 128)[:, :, sigma].reshape(128, 512)
        )
    w2_host = W2.transpose(1, 0, 2).reshape(128, 32 * 128)   # [p, 128g + m]
    # Stage C reads t2p with partition order sigma(k) = 32s+g (k = 4g+s);
    # reorder G's rows to match so the contraction pairs rows correctly.
    g_host = G_pack.transpose(1, 0, 2).reshape(128, 512)     # [k, ch*256 + b]
    g_host = g_host[sigma]
    return (
        f_host.astype(np.float32),
        w2_host.astype(np.float32),
        g_host.astype(np.float32),
    )


_NC_CACHE = {}
_ONES = np.ones((1, 128), dtype=np.float32)


def _build_module(skip_permutes=False, repeat=1, perm_mode="3way", mid_bf16=True,
                  io_on_scalar=False, y_bf16=True, psum_bufs=(2, 3, 3), mid_bufs=6,
                  bias_in_evac=True, strided_perm=True, permb_dram=True):
    """Build + compile the per-core Bass module (cached)."""
    key = ("nc", skip_permutes, repeat, perm_mode, mid_bf16, io_on_scalar, y_bf16,
           psum_bufs, mid_bufs, bias_in_evac, strided_perm, permb_dram)
    if key in _NC_CACHE:
        return _NC_CACHE[key]

    import concourse.bass as bass  # noqa: F401
    import concourse.mybir as mybir
    import concourse.tile as tile
    from concourse import bacc

    f32 = mybir.dt.float32
    f32r = mybir.dt.float32r
    bf16 = mybir.dt.bfloat16
    mid_dt = bf16 if mid_bf16 else f32r
    ps_dt = bf16 if mid_bf16 else f32

    nc = bacc.Bacc("TRN2", target_bir_lowering=False, debug=False)

    xt_d = nc.dram_tensor(
        "xt", [N_CHUNKS, 128, 32, CHUNK], mid_dt, kind="ExternalInput"
    )
    f_d = nc.dram_tensor("fw", [128, 512], mid_dt, kind="ExternalInput")
    w2_d = nc.dram_tensor("w2", [128, 4096], mid_dt, kind="ExternalInput")
    g_d = nc.dram_tensor("gw", [128, 512], mid_dt, kind="ExternalInput")
    bias_d = nc.dram_tensor("biasr", [128, 32], mid_dt, kind="ExternalInput")
    y_dt = (bf16 if mid_bf16 else f32) if y_bf16 else f32
    y_d = nc.dram_tensor("y", [IN_F, TOK_PER_CORE], y_dt, kind="ExternalOutput")
    t2d = nc.dram_tensor("t2d", [2, 128, 8192], mid_dt)

    with tile.TileContext(nc) as tc:
        with (
            tc.tile_pool(name="wpool", bufs=1) as wpool,
            tc.tile_pool(name="pin", bufs=2) as pin,
            tc.tile_pool(name="mid", bufs=mid_bufs) as mid,
            tc.tile_pool(name="psA", bufs=psum_bufs[0], space="PSUM") as psA,
            tc.tile_pool(name="psB", bufs=psum_bufs[1], space="PSUM") as psB,
            tc.tile_pool(name="psC", bufs=psum_bufs[2], space="PSUM") as psC,
        ):
            f_sb = wpool.tile([128, 512], mid_dt, tag="fw")
            w2_sb = wpool.tile([128, 4096], mid_dt, tag="w2")
            g_sb = wpool.tile([128, 512], mid_dt, tag="gw")
            bias_sb = wpool.tile([128, 32], mid_dt, tag="bias")
            nc.sync.dma_start(out=f_sb[:], in_=f_d[:])
            nc.sync.dma_start(out=w2_sb[:], in_=w2_d[:])
            nc.sync.dma_start(out=g_sb[:], in_=g_d[:])
            nc.sync.dma_start(out=bias_sb[:], in_=bias_d[:])


            if perm_mode == "3way":
                # HWDGE only (SP/Act rings, RTL descriptor-gen). Pool DMAs go
                # through SWDGE: the Q7 core emits descriptors in software at
                # ~1us+ per DMA, serialized -- measurably slower for the many
                # small permute DMAs.
                _PERM_PAT = [nc.sync, nc.scalar]
            elif perm_mode == "sync":
                _PERM_PAT = [nc.sync]
            else:
                raise ValueError(perm_mode)
            perm_n = [0]

            def perm_eng(g):
                e = _PERM_PAT[perm_n[0] % len(_PERM_PAT)]
                perm_n[0] += 1
                return e

            evac_n = [0]

            def evac(dst, srcp):
                # A/B evacuation: Act-heavy (DVE also carries stage-C evacs)
                if evac_n[0] % 5 in (0, 3):
                    nc.vector.tensor_copy(dst, srcp)
                else:
                    nc.scalar.copy(dst, srcp)
                evac_n[0] += 1

            def evac_c(dst, srcp, bias_ap):
                # stage-C evacuation with fused bias add (DVE only: Act has no
                # tensor_tensor, Pool cannot access PSUM)
                nc.vector.tensor_tensor(
                    dst, srcp, bias_ap, op=mybir.AluOpType.add
                )

            # ---- pipeline stage bodies (one chunk each) ----
            def st_load(ci, _):
                # split in halves so stage A's first matmuls start earlier
                xts = pin.tile([128, 8192], mid_dt, tag="pin")
                io_eng = nc.scalar if io_on_scalar else nc.sync
                xv = xts[:].rearrange("p (f t) -> p f t", f=32)
                for h in range(2):
                    io_eng.dma_start(
                        out=xv[:, 16 * h: 16 * h + 16],
                        in_=xt_d[ci % N_CHUNKS, :, 16 * h: 16 * h + 16],
                    )
                return xts

            def st_a(ci, xts):
                # stage A: rfft per in-block; t1[k, (16ch+i)*256+t].
                # xt layout is bh-major (f' = 16bh + i) so one 512-col matmul
                # covers an i-pair and the stationary F slice is reused.
                t1 = mid.tile([128, 8192], mid_dt, tag="mid")
                for ch in range(2):
                    for i0 in range(0, NB, 2):
                        ps = psA.tile([128, 512], f32, tag="psA")
                        for bh in range(2):
                            nc.tensor.matmul(
                                ps[:],
                                f_sb[:, bh * 256 + ch * 128: bh * 256 + ch * 128 + 128],
                                xts[:, (16 * bh + i0) * 256: (16 * bh + i0) * 256 + 512],
                                start=(bh == 0),
                                stop=(bh == 1),
                            )
                        q1 = 16 * ch + i0
                        evac(t1[:, q1 * 256: q1 * 256 + 512], ps[:])
                return t1

            def st_pa(ci, t1):
                # permute A->B: t1p[32s+q, 256g+t] = t1[pi(4g+s), 256q+t].
                # strided: pi = sigma (read stride-32 partitions, 4 octets);
                # else pi = identity (read 4 contiguous partitions).
                if skip_permutes:
                    return t1
                t1p = mid.tile([128, 8192], mid_dt, tag="mid")
                if strided_perm:
                    t1r = t1[:].rearrange("(s r) x -> r s x", s=4)
                    for g in range(32):
                        perm_eng(g).dma_start(
                            out=t1p[:, g * 256: g * 256 + 256],
                            in_=t1r[g],
                        )
                else:
                    t1v = t1[:].rearrange("p (q m) -> p q m", m=CHUNK)
                    for g in range(32):
                        perm_eng(g).dma_start(
                            out=t1p[:, g * 256: g * 256 + 256],
                            in_=t1v[4 * g: 4 * g + 4],
                        )
                return t1p

            def st_b(ci, t1p):
                # stage B: per-slot complex mixing; t2 partition = 4q+s
                t2 = mid.tile([128, 8192], mid_dt, tag="mid")
                for g0 in range(0, 32, 2):
                    ps = psB.tile([128, 512], f32, tag="psB")
                    for gg in range(g0, g0 + 2):
                        off = (gg - g0) * 256
                        nc.tensor.matmul(
                            ps[:, off: off + 256],
                            w2_sb[:, gg * 128: gg * 128 + 128],
                            t1p[:, gg * 256: gg * 256 + 256],
                            start=True,
                            stop=True,
                        )
                    evac(t2[:, g0 * 256: g0 * 256 + 512], ps[:])
                return t2

            def st_pb(ci, t2):
                # permute B->C: t2p[32s+g, 256q+t] = t2[rho(q,s), 256g+t].
                # strided: rho = 32s+q (stride-32 reads); else rho = 4q+s.
                if skip_permutes:
                    return t2
                t2p = mid.tile([128, 8192], mid_dt, tag="mid")
                if permb_dram:
                    # DRAM round-trip: runs on the HBM channel in parallel
                    # with the A->B permute's SBUF crossbar traffic.
                    td = t2d[ci % 2]
                    nc.sync.dma_start(out=td, in_=t2[:])
                    if strided_perm:
                        tdv = td.rearrange("(s q) (g t) -> s g q t", s=4, g=32)
                    else:
                        tdv = td.rearrange("(q s) (g t) -> s g q t", s=4, g=32)
                    for s4 in range(4):
                        perm_eng(s4).dma_start(
                            out=t2p[32 * s4: 32 * s4 + 32, :],
                            in_=tdv[s4],
                        )
                elif strided_perm:
                    t2r = t2[:].rearrange("(s r) x -> r s x", s=4)
                    for q in range(32):
                        perm_eng(q).dma_start(
                            out=t2p[:, q * 256: q * 256 + 256],
                            in_=t2r[q],
                        )
                else:
                    t2v = t2[:].rearrange("p (q m) -> p q m", m=CHUNK)
                    for q in range(32):
                        perm_eng(q).dma_start(
                            out=t2p[:, q * 256: q * 256 + 256],
                            in_=t2v[4 * q: 4 * q + 4],
                        )
                return t2p

            def st_c(ci, t2p):
                # stage C flipped: G stationary, t2p moving (512-col j-pairs),
                # feature-major output ysb[b', (bh, j, t)] + fused bias add.
                ysb = mid.tile([128, 8192], mid_dt, tag="mid")
                for bh in range(2):
                    for j0 in range(0, NB, 2):
                        ps = psC.tile([128, 512], f32, tag="psC")
                        for ch in range(2):
                            q4 = 16 * ch + j0
                            nc.tensor.matmul(
                                ps[:],
                                g_sb[:, ch * 256 + bh * 128: ch * 256 + bh * 128 + 128],
                                t2p[:, q4 * 256: q4 * 256 + 512],
                                start=(ch == 0),
                                stop=(ch == 1),
                            )
                        dst = ysb[
                            :, bh * 4096 + j0 * 256: bh * 4096 + j0 * 256 + 512
                        ]
                        evac_c(
                            dst.rearrange("p (j t) -> p j t", j=2),
                            ps[:].rearrange("p (j t) -> p j t", j=2),
                            bias_sb[:, 16 * bh + j0: 16 * bh + j0 + 2]
                            .unsqueeze(2).to_broadcast([128, 2, CHUNK]),
                        )
                return ysb

            def st_store(ci, ysb):
                # feature-major store: y_d[(bh,j) row-block, b', tok-chunk];
                # split by bh halves so each starts as soon as its evacs land
                y_eng = nc.sync if y_bf16 else nc.gpsimd
                c0 = (ci % N_CHUNKS) * 256
                yv = y_d[:].rearrange("(r b) n -> r b n", b=128)
                for bh in range(2):
                    y_eng.dma_start(
                        out=yv[16 * bh: 16 * bh + 16, :, c0: c0 + 256]
                        .transpose([1, 0, 2]),
                        in_=ysb[:, bh * 4096: bh * 4096 + 4096],
                    )
                return None

            # ---- software-pipelined emission: stage s of chunk n at step n+s.
            # Engine streams then interleave A(n) / B(n-1) / C(n-2), so no
            # engine stalls on the evac+permute chain of its own chunk.
            STAGES = [st_load, st_a, st_pa, st_b, st_pb, st_c, st_store]
            n_total = N_CHUNKS * repeat
            state = {}
            for step in range(n_total + len(STAGES) - 1):
                for si in range(len(STAGES) - 1, -1, -1):
                    ci = step - si
                    if 0 <= ci < n_total:
                        state[ci] = STAGES[si](ci, state.get(ci))

    nc.compile()
    _NC_CACHE[key] = nc
    return nc


def kernel(x: np.ndarray, c: np.ndarray, bias: np.ndarray) -> np.ndarray:
    from concourse.bass_utils import run_bass_kernel_spmd

    batch, seq, in_f = x.shape
    n_tok = batch * seq
    xf = np.ascontiguousarray(x.reshape(n_tok, in_f).astype(np.float32))

    f_host, w2_host, g_host = _build_weights(np.asarray(c, dtype=np.float32),
                                             strided=True)
    bias_host = np.ascontiguousarray(
        np.asarray(bias, dtype=np.float32)
        .reshape(NB, 2, 128).transpose(2, 1, 0).reshape(128, 32))
    if MID_BF16:
        f_host = f_host.astype(_BF16)
        w2_host = w2_host.astype(_BF16)
        g_host = g_host.astype(_BF16)
        bias_host = bias_host.astype(_BF16)

    nc = _build_module(mid_bf16=MID_BF16)

    in_maps = []
    for core in range(N_CORES):
        shard = xf[core * TOK_PER_CORE:(core + 1) * TOK_PER_CORE]  # [1024, 4096]
        # xt[ci, p, 16*bh + i, t] = shard[ci*256 + t, 256*i + 128*bh + p]
        xt = np.ascontiguousarray(
            shard.reshape(N_CHUNKS, CHUNK, NB, 2, 128)
            .transpose(0, 4, 3, 2, 1).reshape(N_CHUNKS, 128, 32, CHUNK)
        )
        if MID_BF16:
            xt = xt.astype(_BF16)
        in_maps.append(
            {
                "xt": xt,
                "fw": f_host,
                "w2": w2_host,
                "gw": g_host,
                "biasr": bias_host,
            }
        )

    res = run_bass_kernel_spmd(nc, in_maps, core_ids=list(range(N_CORES)))
    # y_d rows are (bh, j, b') feature-major; unpack to [tok, 256j+128bh+b']
    y = np.concatenate(
        [
            np.asarray(r["y"], dtype=np.float32)
            .reshape(2, NB, 128, TOK_PER_CORE)
            .transpose(3, 1, 0, 2)
            .reshape(TOK_PER_CORE, IN_F)
            for r in res.results
        ],
        axis=0,
    )
    return y.reshape(batch, seq, in_f).astype(x.dtype)

